# revision 48
# baseline (speedup 1.0000x reference)
"""GAT message-passing GNN on 8 Trainium2 NeuronCores (Bass/Tile), v3.

Nodes are permuted (degree-balanced round-robin over 160 tiles of 125) and
partitioned across 8 cores (20 dst tiles each). Per layer a gather table
(512B bf16 row per node: [h bf16(128) | al_src f32(4) | al_dst f32(4) |
pad]) is built distributed (each core its own 20 tiles) and AllGathered.
Per dst tile: one dma_gather fetches the incident edges' src rows; the
static one-hot scatter matrices S4 [e,d] and their transposes SbT [d,e]
stream in from DRAM; al_dst per edge comes from a tiny PE matmul
(SbT^T @ al_dst_own); ew = exp(leakyrelu(al_src+al_dst)) runs on DVE+Act in
f32. Per 128-edge block the weighted one-hot Sw_h = S*ew_h is built with 3
DVE tensor_scalar (4x bf16 mode) + 1 Act scaled-copy, then a single bf16
matmul X^T @ [Sw_0|..|Sw_3] accumulates the transposed per-head aggregation
in PSUM; z comes from N=1 ones-matmuls. The tail folds the 1/4 head-mean
into W, normalizes by z during the head combine (split across DVE/Act),
applies bias+relu+residual, and emits the next table row. Final mean-pool
is a one-hot matmul + AllReduce, then the MLP.
"""
import numpy as np

import concourse.bass as bass
import concourse.bacc as bacc
import concourse.mybir as mybir
import concourse.tile as tile
from concourse.bass_utils import run_bass_kernel_spmd

F32 = mybir.dt.float32
F32R = mybir.dt.float32r
BF16 = mybir.dt.bfloat16
I16 = mybir.dt.int16
AF = mybir.ActivationFunctionType
ALU = mybir.AluOpType

N, E, FIN, HID, HEADS, L, G = 20000, 200000, 20, 128, 4, 4, 32
NEG = 0.2
NCORE = 8
NT = 160            # global dst tiles
TPC = NT // NCORE   # 20 tiles per core
TILE_N = N // NT    # 125 real nodes per tile
PN = NT * 128       # padded node id space
ROW = 256           # bf16 table row: [h(128) | al_src f32(4) | al_dst f32(4) | pad]
PADDST = 999.0

_ZERO_WAIT_OPCODES = (
    "InstDMAGatherAnt",
    "InstDMAScatterAddAnt",
    "InstPartitionBroadcast",
    "InstPartitionAllReduce",
    "InstAPGather",
    "InstIndirectCopy",
    "InstSparseGather",
    "InstGatherTranspose",
)
_spill_counter = [0]


def _split_waits(nc, default_limit=1):
    """Spill excess semaphore waits onto preceding same-engine EventSemaphore
    instructions (walrus wait-slot limits: 0 for extended DMA ops, ~1+ else)."""
    for f in nc.m.functions:
        for bb in f.blocks:
            out = []
            changed = False
            for ins in bb.instructions:
                si = ins.sync_info
                waits = list(si.on_wait) if si is not None and si.on_wait else []
                tname = type(ins).__name__
                limit = default_limit
                if tname in _ZERO_WAIT_OPCODES:
                    limit = 0
                elif ins.engine == mybir.EngineType.Pool and tname in (
                    "InstDrain",
                    "InstNoOp",
                ):
                    limit = 0
                if len(waits) > limit:
                    changed = True
                    keep = waits[:limit] if limit else []
                    spill = waits[limit:] if limit else waits
                    while spill:
                        chunk, spill = spill[:1], spill[1:]
                        _spill_counter[0] += 1
                        nop = mybir.InstEventSemaphore(
                            name=f"waitspill-{_spill_counter[0]}"
                        )
                        nop.engine = ins.engine
                        nop.sync_info = mybir.SyncInfo(on_wait=chunk, on_update=[])
                        nc.register_instruction(nop, overwrite=True)
                        out.append(nop)
                    ins.sync_info = mybir.SyncInfo(
                        on_wait=keep, on_update=list(si.on_update) if si else []
                    )
                out.append(ins)
            if changed:
                bb.instructions[:] = out


def _preprocess(x, edge_index, batch, gat_W, att_src, att_dst):
    """Degree-balanced node permutation + per-core edge/tile data."""
    bt = mybir.dt.np(BF16)
    src = np.concatenate([edge_index[0], np.arange(N, dtype=np.int64)])
    dst = np.concatenate([edge_index[1], np.arange(N, dtype=np.int64)])
    indeg = np.bincount(dst, minlength=N)
    order = np.argsort(-indeg, kind="stable")
    new_id = np.empty(N, dtype=np.int64)
    ranks = np.arange(N)
    new_id[order] = (ranks % NT) * 128 + (ranks // NT)

    nsrc = new_id[src]
    ndst = new_id[dst]
    tile_e = ndst >> 7
    dloc = ndst & 127
    eorder = np.argsort(tile_e, kind="stable")
    tile_sorted = tile_e[eorder]
    nsrc_sorted = nsrc[eorder]
    dloc_sorted = dloc[eorder]
    starts = np.searchsorted(tile_sorted, np.arange(NT + 1))
    cnts = np.diff(starts)
    nblk = int(np.ceil(cnts.max() / 128))
    ET = nblk * 128

    gsrc = np.zeros((NT, ET), dtype=np.int64)       # src row ids (pad -> 0)
    gdst = np.full((NT, ET), int(PADDST), dtype=np.int64)  # dst local col
    for t in range(NT):
        s, c = starts[t], cnts[t]
        gsrc[t, :c] = nsrc_sorted[s : s + c]
        gdst[t, :c] = dloc_sorted[s : s + c]

    # per-core arrays
    gidx = np.zeros((NCORE, 128, TPC * nblk * 8), dtype=np.int16)
    S4 = np.zeros((NCORE, 128, TPC * nblk * 128), dtype=bt)
    SbT = np.zeros((NCORE, 128, TPC * nblk * 128), dtype=bt)
    dmat = np.arange(128)
    for c in range(NCORE):
        for tl in range(TPC):
            t = c * TPC + tl
            wrap_s = gsrc[t].astype(np.int16).reshape(ET // 16, 16).T
            gidx[c, :, tl * nblk * 8 : (tl + 1) * nblk * 8] = np.tile(wrap_s, (8, 1))
            oh = (gdst[t][:, None] == dmat[None, :]).astype(np.float32)  # [ET, 128]
            ohb = oh.reshape(nblk, 128, 128)
            S4[c, :, tl * nblk * 128 : (tl + 1) * nblk * 128] = (
                ohb.transpose(1, 0, 2).reshape(128, nblk * 128).astype(bt)
            )
            SbT[c, :, tl * nblk * 128 : (tl + 1) * nblk * 128] = (
                ohb.transpose(2, 0, 1).reshape(128, nblk * 128).astype(bt)
            )

    # pooling matrix with 1/cnt folded in
    cnt = np.bincount(batch, minlength=G).astype(np.float32)
    cnt = np.maximum(cnt, 1.0)
    btile = np.zeros((NCORE, 128, TPC * 32), dtype=np.float32)
    inv = np.zeros(PN, dtype=np.int64)
    inv[new_id] = np.arange(N)
    valid = np.zeros(PN, dtype=bool)
    valid[new_id] = True
    for c in range(NCORE):
        for tl in range(TPC):
            t = c * TPC + tl
            for p in range(TILE_N):
                nid = t * 128 + p
                if valid[nid]:
                    g = batch[inv[nid]]
                    btile[c, p, tl * 32 + g] = 1.0 / cnt[g]

    # permuted transposed input features, per-core slices, bf16
    xT = np.zeros((FIN, PN), dtype=np.float32)
    xT[:, new_id] = x.T
    xob = np.stack(
        [xT[:, c * TPC * 128 : (c + 1) * TPC * 128].astype(bt) for c in range(NCORE)]
    )

    # folded attention projections WA_l = W_l @ [A_src | A_dst]
    WA = np.zeros((L, HID, 2 * HEADS), dtype=np.float32)
    for l in range(L):
        A = np.zeros((HID * HEADS, 2 * HEADS), dtype=np.float64)
        for h in range(HEADS):
            A[h * HID : (h + 1) * HID, h] = att_src[l][h]
            A[h * HID : (h + 1) * HID, HEADS + h] = att_dst[l][h]
        WA[l] = (gat_W[l].astype(np.float64) @ A).astype(np.float32)

    return dict(
        gidx=gidx,
        S4=S4,
        SbT=SbT,
        btile=btile,
        xob=xob,
        WA=WA,
        nblk=nblk,
    )


def _build(nblk, dbg_stop=None, dbg_layers=L, dbg_pad=0, dbg_dump=None):
    nc = bacc.Bacc("TRN2", target_bir_lowering=False, debug=False, num_devices=NCORE)

    t_xob = nc.dram_tensor("xob", [FIN, TPC * 128], BF16, kind="ExternalInput")
    t_gidx = nc.dram_tensor("gidx", [128, TPC * nblk * 8], I16, kind="ExternalInput")
    t_S4 = nc.dram_tensor("S4", [128, TPC * nblk * 128], BF16, kind="ExternalInput")
    t_SbT = nc.dram_tensor("SbT", [128, TPC * nblk * 128], BF16, kind="ExternalInput")
    t_onesb = nc.dram_tensor("onesb", [128, 1], BF16, kind="ExternalInput")
    t_iotac = nc.dram_tensor("iotac", [128, 1], F32, kind="ExternalInput")
    t_identb = nc.dram_tensor("identb", [128, 128], BF16, kind="ExternalInput")
    t_ident = nc.dram_tensor("ident", [128, 128], F32, kind="ExternalInput")
    t_btileb = nc.dram_tensor("btileb", [128, TPC * 32], BF16, kind="ExternalInput")
    t_Winb = nc.dram_tensor("Winb", [FIN, HID], BF16, kind="ExternalInput")
    t_Wqb = nc.dram_tensor("Wqb", [128, L * HEADS * 128], BF16, kind="ExternalInput")
    t_WAb = nc.dram_tensor("WAb", [128, L * 2 * HEADS], BF16, kind="ExternalInput")
    t_btl = nc.dram_tensor("btl", [128, L * HID], F32, kind="ExternalInput")
    t_W1 = nc.dram_tensor("W1", [HID, 64], F32R, kind="ExternalInput")
    t_W2 = nc.dram_tensor("W2", [64, 64], F32R, kind="ExternalInput")
    t_W3 = nc.dram_tensor("W3", [64, 32], F32R, kind="ExternalInput")
    t_b1 = nc.dram_tensor("b1t", [32, 64], F32, kind="ExternalInput")
    t_b2 = nc.dram_tensor("b2t", [32, 64], F32, kind="ExternalInput")
    t_b3 = nc.dram_tensor("b3t", [32, 32], F32, kind="ExternalInput")
    o_out = nc.dram_tensor("out", [G, 32], F32, kind="ExternalOutput")
    o_hd = (
        nc.dram_tensor("hd", [128, TPC * HID], BF16, kind="ExternalOutput")
        if dbg_dump is not None
        else None
    )
    o_ad = (
        nc.dram_tensor("ad", [128, TPC * HEADS], BF16, kind="ExternalOutput")
        if dbg_dump is not None
        else None
    )
    o_ew = (
        nc.dram_tensor("ewd", [128, 3 * nblk * HEADS], F32, kind="ExternalOutput")
        if dbg_dump is not None
        else None
    )
    o_rawt = (
        nc.dram_tensor("rawt", [128, 16], BF16, kind="ExternalOutput")
        if dbg_dump is not None
        else None
    )
    o_rawx = (
        nc.dram_tensor("rawx", [128, 16], BF16, kind="ExternalOutput")
        if dbg_dump is not None
        else None
    )
    o_zc = (
        nc.dram_tensor("zcd", [128, HEADS], F32, kind="ExternalOutput")
        if dbg_dump is not None
        else None
    )
    o_pso = (
        nc.dram_tensor("psod", [128, HEADS * 128], BF16, kind="ExternalOutput")
        if dbg_dump is not None
        else None
    )
    o_hn = (
        nc.dram_tensor("hnd", [128, 4 * HID], F32, kind="ExternalOutput")
        if dbg_dump is not None
        else None
    )
    o_sw = (
        nc.dram_tensor("swd", [128, HEADS * 128], BF16, kind="ExternalOutput")
        if dbg_dump is not None
        else None
    )

    with tile.TileContext(nc) as tc:
        with (
            tc.tile_pool(name="const", bufs=1) as cpool,
            tc.tile_pool(name="persist", bufs=1) as hpool,
            tc.tile_pool(name="dram", bufs=1, space="DRAM") as dpool,
        ):
            onesb = cpool.tile([128, 1], BF16)
            iotac = cpool.tile([128, 1], F32)
            nc.sync.dma_start(iotac[:], t_iotac[:])
            identb = cpool.tile([128, 128], BF16)
            ident = cpool.tile([128, 128], F32)
            Winb = cpool.tile([FIN, HID], BF16)
            Wqb = cpool.tile([128, L, HEADS, 128], BF16)
            WAb = cpool.tile([128, L, 2 * HEADS], BF16)
            btl = cpool.tile([128, L, HID], F32)
            btileb = cpool.tile([128, TPC * 32], BF16)
            gidx = cpool.tile([128, TPC * nblk * 8], I16)
            nc.sync.dma_start(onesb[:], t_onesb[:])
            nc.sync.dma_start(identb[:], t_identb[:])
            nc.sync.dma_start(ident[:], t_ident[:])
            nc.sync.dma_start(Winb[:], t_Winb[:])
            nc.sync.dma_start(
                Wqb[:], t_Wqb[:].rearrange("p (l h d) -> p l h d", l=L, h=HEADS)
            )
            nc.sync.dma_start(WAb[:], t_WAb[:].rearrange("p (l a) -> p l a", l=L))
            nc.sync.dma_start(btl[:], t_btl[:].rearrange("p (l f) -> p l f", l=L))
            nc.sync.dma_start(btileb[:], t_btileb[:])
            nc.sync.dma_start(gidx[:], t_gidx[:])

            h_old_own = hpool.tile([128, TPC, HID], BF16)
            al_dst_own = hpool.tile([128, TPC, HEADS], BF16)

            cc_in = dpool.tile([TPC, 128, ROW], BF16)
            ags = [
                dpool.tile([NT, 128, ROW], BF16, addr_space="Shared", name=f"ag{i}")
                for i in range(L)
            ]
            ar_in = dpool.tile([G, HID], F32)
            ar_out = dpool.tile([G, HID], F32, addr_space="Shared")

            if dbg_pad:
                with tc.tile_pool(name="padp", bufs=2) as padp:
                    pa = padp.tile([1, 16], F32, tag="pa")
                    nc.vector.memset(pa[:], 0.0)
                    for _ in range(dbg_pad):
                        pb = padp.tile([1, 16], F32, tag="pa")
                        nc.vector.tensor_copy(pb[:], pa[:])
                        pa = pb

            # ---- phase 0: own tiles' h0/al0 -> cc rows ----
            with (
                tc.tile_pool(name="p0s", bufs=2) as p0s,
                tc.tile_pool(name="p0p", bufs=2, space="PSUM") as p0p,
            ):
                for g0 in range(0, TPC, 4):
                    xt = p0s.tile([FIN, 4, 128], BF16, tag="xt")
                    nc.sync.dma_start(
                        xt[:],
                        t_xob[:, g0 * 128 : (g0 + 4) * 128].rearrange(
                            "f (j n) -> f j n", j=4
                        ),
                    )
                    phc4 = p0p.tile([HID, 4, 128], F32, tag="phc")
                    nc.tensor.matmul(
                        phc4[:].rearrange("f j n -> f (j n)"),
                        Winb[:],
                        xt[:].rearrange("f j n -> f (j n)"),
                        start=True,
                        stop=True,
                    )
                    h0T4 = p0s.tile([HID, 4, 128], BF16, tag="h0T")
                    nc.scalar.activation(h0T4[:], phc4[:], AF.Relu)
                    ph4 = p0p.tile([128, 4, HID], BF16, tag="ph4")
                    al84 = p0p.tile([128, 4, 2 * HEADS], F32, tag="al84")
                    for j in range(4):
                        nc.tensor.transpose(ph4[:, j, :], h0T4[:, j, :], identb[:])
                        nc.tensor.matmul(
                            al84[:, j, :],
                            h0T4[:, j, :],
                            WAb[:, 0, :],
                            start=True,
                            stop=True,
                        )
                    stage4 = p0s.tile([128, 4, ROW], BF16, tag="st4")
                    nc.vector.tensor_copy(stage4[:, 0:2, 0:HID], ph4[:, 0:2, :])
                    nc.scalar.activation(
                        stage4[:, 2:4, 0:HID], ph4[:, 2:4, :], AF.Copy
                    )
                    nc.vector.tensor_copy(
                        stage4[:, :, HID : HID + 16].bitcast(F32), al84[:]
                    )
                    nc.vector.tensor_copy(
                        h_old_own[:, g0 : g0 + 4, :], stage4[:, :, 0:HID]
                    )
                    nc.vector.tensor_copy(
                        al_dst_own[:, g0 : g0 + 4, :], al84[:, :, HEADS : 2 * HEADS]
                    )
                    if dbg_dump is not None:
                        for j in range(4):
                            nc.vector.tensor_scalar(
                                stage4[:, j, 144:146].bitcast(F32),
                                iotac[:],
                                float((g0 + j) * 128),
                                None,
                                ALU.add,
                            )
                    nc.sync.dma_start(
                        cc_in[g0 : g0 + 4].rearrange("j p c -> p j c"), stage4[:]
                    )

            if dbg_stop == "p0":
                return nc
            if dbg_stop not in ("nocc", "sim"):
                nc.gpsimd.collective_compute(
                    "AllGather",
                    ALU.bypass,
                    replica_groups=[list(range(NCORE))],
                    ins=[cc_in[:, :, :].opt()],
                    outs=[ags[0][:, :, :].opt()],
                )

            for l in range(dbg_layers):
                with (
                    tc.tile_pool(name="p2s", bufs=2) as p2s,
                    tc.tile_pool(name="p2p", bufs=2, space="PSUM") as p2p,
                ):
                    for tl in range(TPC):
                        X2 = p2s.tile([128, nblk, ROW], BF16, tag="X", bufs=4)
                        gsrc_t = ags[l][:, :, :].rearrange("t n c -> (t n) c")
                        for g0 in range(0, nblk, 8):
                            g1 = min(g0 + 8, nblk)
                            nc.gpsimd.dma_gather(
                                X2[:, g0:g1, :],
                                gsrc_t,
                                gidx[
                                    :,
                                    tl * nblk * 8 + g0 * 8 : tl * nblk * 8 + g1 * 8,
                                ],
                                (g1 - g0) * 128,
                                (g1 - g0) * 128,
                                ROW,
                            )
                        S4 = p2s.tile([128, nblk, 128], BF16, tag="S4", bufs=4)
                        nc.sync.dma_start(
                            S4[:],
                            t_S4[
                                :, tl * nblk * 128 : (tl + 1) * nblk * 128
                            ].rearrange("e (b d) -> e b d", b=nblk),
                        )
                        SbT = p2s.tile([128, nblk, 128], BF16, tag="SbT", bufs=4)
                        nc.sync.dma_start(
                            SbT[:],
                            t_SbT[
                                :, tl * nblk * 128 : (tl + 1) * nblk * 128
                            ].rearrange("d (b e) -> d b e", b=nblk),
                        )
                        # al_dst per edge via SbT^T @ al_dst_own
                        aldp = p2p.tile([128, nblk, HEADS], F32, tag="aldp", bufs=1)
                        for b in range(nblk):
                            nc.tensor.matmul(
                                aldp[:, b, :],
                                SbT[:, b, :],
                                al_dst_own[:, tl, :],
                                start=True,
                                stop=True,
                            )
                        # ew = exp(leakyrelu(al_src + al_dst))  (f32)
                        ewp = p2s.tile([128, nblk, HEADS], F32, tag="ewp")
                        nc.vector.tensor_add(
                            ewp[:],
                            X2[:, :, HID : HID + 8].bitcast(F32)[:, :, 0:4],
                            aldp[:],
                        )
                        ewl = p2s.tile([128, nblk, HEADS], F32, tag="ewl")
                        nc.vector.scalar_tensor_tensor(
                            ewl[:], ewp[:], NEG, ewp[:], ALU.mult, ALU.max
                        )
                        ewb = p2s.tile([128, nblk * HEADS], F32, tag="ewb")
                        nc.scalar.activation(
                            ewb[:].rearrange("e (b h) -> e b h", b=nblk),
                            ewl[:],
                            AF.Exp,
                        )
                        ewbb = p2s.tile([128, nblk, HEADS], BF16, tag="ewbb")
                        nc.vector.tensor_copy(
                            ewbb[:], ewb[:].rearrange("e (b h) -> e b h", b=nblk)
                        )
                        if dbg_dump is not None and l == 0 and tl == 0:
                            dbgt = p2s.tile([128, 3, nblk * HEADS], F32, tag="dbgt")
                            nc.vector.tensor_copy(
                                dbgt[:, 0, 0:nblk],
                                X2[:, :, 144:146].bitcast(F32)[:, :, 0],
                            )
                            nc.vector.tensor_copy(
                                dbgt[:, 0, nblk : nblk * HEADS].rearrange(
                                    "e (b h) -> e b h", h=3
                                ),
                                X2[:, :, HID : HID + 8].bitcast(F32)[:, :, 0:3],
                            )
                            nc.vector.tensor_copy(
                                dbgt[:, 1, :].rearrange("e (b h) -> e b h", b=nblk),
                                aldp[:],
                            )
                            nc.vector.tensor_copy(dbgt[:, 2, :], ewb[:])
                            rawt = p2s.tile([128, 16], BF16, tag="rawt")
                            nc.sync.dma_start(
                                rawt[:], ags[l][0, :, HID : HID + 16]
                            )
                            nc.sync.dma_start(o_rawt[:], rawt[:])
                            rawx = p2s.tile([128, 16], BF16, tag="rawx")
                            nc.vector.tensor_copy(
                                rawx[:], X2[:, 0, HID : HID + 16]
                            )
                            nc.sync.dma_start(o_rawx[:], rawx[:])
                            nc.sync.dma_start(
                                o_ew[:].rearrange("e (k c) -> e k c", k=3),
                                dbgt[:],
                            )

                        poutT = p2p.tile([HID, HEADS, 128], F32, tag="poutT", bufs=2)
                        pz = p2p.tile([128, HEADS], F32, tag="pz", bufs=2)
                        for b in range(nblk):
                            Sw4 = p2s.tile(
                                [128, HEADS, 128], BF16, tag=f"Sw{b % 3}"
                            )
                            for h in range(3):
                                nc.vector.tensor_scalar(
                                    Sw4[:, h, :],
                                    S4[:, b, :],
                                    ewb[:, b * HEADS + h : b * HEADS + h + 1],
                                    None,
                                    ALU.mult,
                                )
                            nc.scalar.activation(
                                Sw4[:, 3, :],
                                S4[:, b, :],
                                AF.Copy,
                                scale=ewb[:, b * HEADS + 3 : b * HEADS + 4],
                            )
                            if dbg_dump is not None and l == 0 and tl == 0 and b == 0:
                                nc.sync.dma_start(
                                    o_sw[:].rearrange("e (h d) -> e h d", h=HEADS),
                                    Sw4[:],
                                )
                            nc.tensor.matmul(
                                poutT[:].rearrange("f h d -> f (h d)"),
                                X2[:, b, 0:HID],
                                Sw4[:].rearrange("e h d -> e (h d)"),
                                start=(b == 0),
                                stop=(b == nblk - 1),
                            )
                            nc.tensor.matmul(
                                pz[:],
                                S4[:, b, :],
                                ewbb[:, b, :],
                                start=(b == 0),
                                stop=(b == nblk - 1),
                            )
                        # z -> reciprocal (0.25 head-mean folded into Wqb)
                        zc = p2s.tile([128, HEADS], F32, tag="zc")
                        nc.vector.tensor_scalar(zc[:], pz[:], 1e-30, None, ALU.max)
                        zr = p2s.tile([128, HEADS], F32, tag="zr")
                        nc.vector.reciprocal(zr[:], zc[:])
                        # poutT -> sbuf (split DVE / Act / Pool)
                        pso = p2s.tile([HID, HEADS, 128], BF16, tag="pso")
                        nc.vector.tensor_copy(pso[:, 0:2, :], poutT[:, 0:2, :])
                        nc.scalar.activation(
                            pso[:, 2:4, :], poutT[:, 2:4, :], AF.Copy
                        )
                        proj = p2p.tile([128, HEADS, HID], F32, tag="proj", bufs=1)
                        for h in range(HEADS):
                            nc.tensor.matmul(
                                proj[:, h, :],
                                pso[:, h, :],
                                Wqb[:, l, h, :],
                                start=True,
                                stop=True,
                            )
                        # hn = relu(sum_h zr_h*proj_h + b) + h_old
                        c1 = p2s.tile([128, HID], F32, tag="c1")
                        nc.vector.scalar_tensor_tensor(
                            c1[:],
                            proj[:, 0, :],
                            zr[:, 0:1],
                            btl[:, l, :],
                            ALU.mult,
                            ALU.add,
                        )
                        c2 = p2s.tile([128, HID], BF16, tag="c2")
                        nc.vector.scalar_tensor_tensor(
                            c2[:], proj[:, 1, :], zr[:, 1:2], c1[:], ALU.mult, ALU.add
                        )
                        a3 = p2s.tile([128, HID], BF16, tag="a3")
                        nc.scalar.activation(
                            a3[:], proj[:, 2, :], AF.Copy, scale=zr[:, 2:3]
                        )
                        a4 = p2s.tile([128, HID], BF16, tag="a4")
                        nc.scalar.activation(
                            a4[:], proj[:, 3, :], AF.Copy, scale=zr[:, 3:4]
                        )
                        c34 = p2s.tile([128, HID], BF16, tag="c34")
                        nc.vector.tensor_add(c34[:], a3[:], a4[:])
                        s1 = p2s.tile([128, HID], BF16, tag="s1")
                        nc.vector.tensor_add(s1[:], c2[:], c34[:])
                        stage = p2s.tile([128, ROW], BF16, tag="stg")
                        nc.vector.scalar_tensor_tensor(
                            stage[:, 0:HID],
                            s1[:],
                            0.0,
                            h_old_own[:, tl, :],
                            ALU.max,
                            ALU.add,
                        )
                        nc.gpsimd.tensor_copy(h_old_own[:, tl, :], stage[:, 0:HID])
                        if dbg_dump is not None and l == 0 and tl == 0:
                            nc.sync.dma_start(o_zc[:], zc[:])
                            nc.sync.dma_start(
                                o_pso[:].rearrange("f (h d) -> f h d", h=HEADS),
                                pso[:],
                            )
                            hnd = p2s.tile([128, 4, HID], F32, tag="hnd")
                            nc.vector.tensor_copy(hnd[:, 0, :], c1[:])
                            nc.vector.tensor_copy(hnd[:, 1, :], c2[:])
                            nc.vector.tensor_copy(hnd[:, 2, :], c34[:])
                            nc.vector.tensor_copy(hnd[:, 3, :], s1[:])
                            nc.sync.dma_start(
                                o_hn[:].rearrange("e (j f) -> e j f", j=4), hnd[:]
                            )
                        if l < L - 1:
                            hnT_ps = p2p.tile([HID, 128], BF16, tag="hnT", bufs=1)
                            nc.tensor.transpose(
                                hnT_ps[:], stage[:, 0:HID], identb[:]
                            )
                            hnT = p2s.tile([HID, 128], BF16, tag="hnTs")
                            nc.scalar.activation(hnT[:], hnT_ps[:], AF.Copy)
                            al8 = p2p.tile([128, 2 * HEADS], F32, tag="al8", bufs=1)
                            nc.tensor.matmul(
                                al8[:], hnT[:], WAb[:, l + 1, :], start=True, stop=True
                            )
                            nc.scalar.activation(
                                stage[:, HID : HID + 16].bitcast(F32),
                                al8[:],
                                AF.Copy,
                            )
                            nc.vector.tensor_copy(
                                al_dst_own[:, tl, :], al8[:, HEADS : 2 * HEADS]
                            )
                            nc.sync.dma_start(cc_in[tl], stage[:])

                if dbg_stop == "p2":
                    break
                if l < L - 1:
                    if dbg_stop in ("nocc", "sim"):
                        continue
                    nc.gpsimd.collective_compute(
                        "AllGather",
                        ALU.bypass,
                        replica_groups=[list(range(NCORE))],
                        ins=[cc_in[:, :, :].opt()],
                        outs=[ags[l + 1][:, :, :].opt()],
                    )

            if dbg_dump is not None:
                with tc.tile_pool(name="dbg", bufs=1) as dbp:
                    zo = dbp.tile([G, 32], F32)
                    nc.vector.memset(zo[:], 0.0)
                    nc.sync.dma_start(o_out[:], zo[:])
                    nc.sync.dma_start(
                        o_hd[:].rearrange("p (t f) -> p t f", t=TPC), h_old_own[:]
                    )
                    nc.sync.dma_start(
                        o_ad[:].rearrange("p (t a) -> p t a", t=TPC), al_dst_own[:]
                    )
                return nc
            if dbg_stop in ("p2", "sim"):
                return nc
            # ---- P4: graph mean pool + MLP ----
            with (
                tc.tile_pool(name="p4s", bufs=2) as p4s,
                tc.tile_pool(name="p4p", bufs=1, space="PSUM") as p4p,
            ):
                ppool = p4p.tile([32, HID], F32, tag="pool")
                for tl in range(TPC):
                    nc.tensor.matmul(
                        ppool[:],
                        btileb[:, tl * 32 : (tl + 1) * 32],
                        h_old_own[:, tl, :],
                        start=(tl == 0),
                        stop=(tl == TPC - 1),
                    )
                pool_sb = p4s.tile([32, HID], F32)
                nc.vector.tensor_copy(pool_sb[:], ppool[:])
                nc.sync.dma_start(ar_in[:], pool_sb[:])
                nc.gpsimd.collective_compute(
                    "AllReduce",
                    ALU.add,
                    replica_groups=[list(range(NCORE))],
                    ins=[ar_in[:].opt()],
                    outs=[ar_out[:].opt()],
                )
                g_sb = p4s.tile([G, HID], F32)
                nc.sync.dma_start(g_sb[:], ar_out[:])

                def t_r(src_ap, pdim, fdim, tag):
                    ps = p4p.tile([fdim, pdim], F32, tag=tag + "p")
                    nc.tensor.transpose(ps[:], src_ap, ident[:pdim, :pdim])
                    sb = p4s.tile([fdim, pdim], F32R, tag=tag)
                    nc.vector.tensor_copy(sb[:], ps[:])
                    return sb

                W1 = p4s.tile([HID, 64], F32R)
                W2 = p4s.tile([64, 64], F32R)
                W3 = p4s.tile([64, 32], F32R)
                b1 = p4s.tile([32, 64], F32)
                b2 = p4s.tile([32, 64], F32)
                b3 = p4s.tile([32, 32], F32)
                nc.sync.dma_start(W1[:], t_W1[:])
                nc.sync.dma_start(W2[:], t_W2[:])
                nc.sync.dma_start(W3[:], t_W3[:])
                nc.sync.dma_start(b1[:], t_b1[:])
                nc.sync.dma_start(b2[:], t_b2[:])
                nc.sync.dma_start(b3[:], t_b3[:])

                gT = t_r(g_sb[:], G, HID, "gT")
                pm1 = p4p.tile([G, 64], F32, tag="m1")
                nc.tensor.matmul(pm1[:], gT[:], W1[:], start=True, stop=True)
                o1 = p4s.tile([G, 64], F32, tag="o1")
                nc.vector.tensor_add(o1[:], pm1[:], b1[:])
                nc.scalar.activation(o1[:], o1[:], AF.Relu)

                o1T = t_r(o1[:], G, 64, "o1T")
                pm2 = p4p.tile([G, 64], F32, tag="m2")
                nc.tensor.matmul(pm2[:], o1T[:], W2[:], start=True, stop=True)
                o2 = p4s.tile([G, 64], F32, tag="o2")
                nc.vector.tensor_add(o2[:], pm2[:], b2[:])
                nc.scalar.activation(o2[:], o2[:], AF.Relu)

                o2T = t_r(o2[:], G, 64, "o2T")
                pm3 = p4p.tile([G, 32], F32, tag="m3")
                nc.tensor.matmul(pm3[:], o2T[:], W3[:], start=True, stop=True)
                o3 = p4s.tile([G, 32], F32, tag="o3")
                nc.vector.tensor_add(o3[:], pm3[:], b3[:])
                nc.sync.dma_start(o_out[:], o3[:])
    return nc


_CACHE = {}
_LAST_NBLK = 11


def _get_program(nblk):
    if nblk not in _CACHE:
        nc = _build(nblk)
        _split_waits(nc)
        nc.compile()
        _CACHE[nblk] = nc
    return _CACHE[nblk]


def kernel(**inputs):
    import os

    bt = mybir.dt.np(BF16)
    inp = {k: np.asarray(v) for k, v in inputs.items()}
    prep = _preprocess(
        inp["x"].astype(np.float32),
        inp["edge_index"].astype(np.int64),
        inp["batch"].astype(np.int64),
        inp["gat_W"].astype(np.float32),
        inp["att_src"].astype(np.float32),
        inp["att_dst"].astype(np.float32),
    )
    nblk = prep["nblk"]
    global _LAST_NBLK
    _LAST_NBLK = nblk
    nc = _get_program(nblk)

    identb = np.eye(128, dtype=np.float32).astype(bt)
    ident = np.eye(128, dtype=np.float32)
    onesb = np.ones((128, 1), np.float32).astype(bt)
    Wq = (
        inp["gat_W"].astype(np.float32).reshape(L, HID, HEADS, HID) / HEADS
    ).transpose(1, 0, 2, 3)
    btlrow = np.tile(inp["gat_b"].astype(np.float32).reshape(L, 1, HID), (1, 128, 1))
    btlrow = btlrow.transpose(1, 0, 2).reshape(128, L * HID)
    b1t = np.tile(inp["b1"].astype(np.float32)[None, :], (32, 1))
    b2t = np.tile(inp["b2"].astype(np.float32)[None, :], (32, 1))
    b3t = np.tile(inp["b3"].astype(np.float32)[None, :], (32, 1))

    shared = dict(
        onesb=onesb,
        iotac=np.zeros((128, 1), np.float32),
        identb=identb,
        ident=ident,
        Winb=inp["W_in"].astype(np.float32).astype(bt),
        Wqb=Wq.reshape(HID, L * HEADS * 128).astype(bt),
        WAb=prep["WA"].transpose(1, 0, 2).reshape(HID, L * 2 * HEADS).astype(bt),
        btl=btlrow,
        W1=inp["W1"].astype(np.float32),
        W2=inp["W2"].astype(np.float32),
        W3=inp["W3"].astype(np.float32),
        b1t=b1t,
        b2t=b2t,
        b3t=b3t,
    )
    in_maps = []
    for c in range(NCORE):
        m = dict(shared)
        m["xob"] = prep["xob"][c]
        m["gidx"] = prep["gidx"][c]
        m["S4"] = prep["S4"][c]
        m["SbT"] = prep["SbT"][c]
        m["btileb"] = prep["btile"][c].astype(bt)
        in_maps.append(m)

    trace = bool(int(os.environ.get("KERNEL_TRACE", "0")))
    last_exc = None
    for attempt in range(3):
        try:
            res = run_bass_kernel_spmd(
                nc, in_maps, core_ids=list(range(NCORE)), trace=trace
            )
            break
        except Exception as exc:  # transient device-unrecoverable after crashes
            last_exc = exc
            import time as _time

            _time.sleep(15)
    else:
        raise last_exc
    if trace and res.exec_time_ns is not None:
        print(f"HW exec time: {res.exec_time_ns} ns")
        kernel.last_exec_time_ns = res.exec_time_ns
        kernel.last_trace = res.instructions_and_trace
    return np.asarray(res.results[0]["out"], dtype=np.float32)
